# revision 24
# baseline (speedup 1.0000x reference)
"""Trainium2 Bass kernel for nn_Block_72146860638363 (Performer/FAVOR+ block).

Sharding: data-parallel over batch B=8, one batch element per NeuronCore,
no collectives. All params + SPE codes replicated per core.

v2 restructure vs baseline:
- fused single-pass double-LN (exact algebra for trivial g/b)
- batched 3D-out DMA transposes (1 instr per token tile instead of 12)
- v-part matmuls fused into stage A per token tile (PE busy from ~5us)
- software-pipelined head loop (head h's FAVOR chains overlap head h+1's
  qkv/SPE matmuls) so the PE FIFO queue never starves
- fc2 weight streaming (quarter-chunks, bufs=5) for true depth-1 prefetch
  across dt iterations (kills both the DMA stall and p-state re-throttle)
- col-paired SPE matmuls: q outputs on psum partitions 0:64, k on 64:128
  (different array column groups -> concurrent on HW); khT uses a
  row-swapped w (wTs) so all psum->SBUF copies stay partition-aligned
- fp8 e4m3 DoubleRow for the q/k qkv part and the output projection
  (numerically free: q,k only steer normalized attention weights);
  v and the MLP stay bf16 (fp8 there fails the 2e-2 budget)
- wide stride-0-broadcast Gq/Gk products (one DVE op per sine per side)
"""
import json
import math
import sys

for _p in ("/opt/trn_rl_repo", "/root/.axon_site/_ro/trn_rl_repo"):
    if _p not in sys.path:
        sys.path.insert(0, _p)

import numpy as np
import ml_dtypes

import concourse.bass as bass
import concourse.bass_isa as bass_isa
import concourse.mybir as mybir
import concourse.tile as tile
from concourse.masks import make_identity

BF16NP = ml_dtypes.bfloat16
E4NP = ml_dtypes.float8_e4m3fn
F32 = mybir.dt.float32
BF16 = mybir.dt.bfloat16
FP8 = mybir.dt.float8e4
QK_SC = 64.0      # fp8 weight scale for the qk and proj gemms
AF = mybir.ActivationFunctionType
ALU = mybir.AluOpType
AX = mybir.AxisListType

B, N, DIM, H, D = 8, 1024, 1536, 12, 128
M, R, S = 64, 64, 5
P = 128
NT = N // P          # 8 token tiles
DC = DIM // P        # 12 feature chunks
FC1 = 4 * DIM        # 6144
OT1 = FC1 // P       # 48
EPS_LN = 1e-5
EPS_K = 1e-4


# ---------------------------------------------------------------------------
# BIR post-processing: this container's walrus rejects >1 sem wait per
# instruction (>2 for EventSemaphore). Split extra waits onto wait-only
# Drain carriers inserted just before, same engine.
def _split_multiwait(bir_json_bytes: bytes) -> bytes:
    m = json.loads(bir_json_bytes)
    changed = False
    for fn in m.get("functions", []):
        for bb in fn.get("blocks", []):
            new_insts = []
            for inst in bb.get("instructions", []):
                si = inst.get("sync_info") or {}
                waits = si.get("on_wait") or []
                cap = 2 if inst.get("opcode") == "EventSemaphore" else 1
                if len(waits) > cap:
                    changed = True
                    for i, w in enumerate(waits[:-cap]):
                        new_insts.append({
                            "debug": inst.get("debug", 0),
                            "engine": inst["engine"],
                            "ins": [],
                            "is_reset_sema": False,
                            "name": f"{inst['name']}_w{i}",
                            "opcode": "Drain",
                            "outs": [],
                            "sync_info": {"on_update": [], "on_wait": [w]},
                        })
                    si["on_wait"] = waits[-cap:]
                new_insts.append(inst)
            bb["instructions"] = new_insts
    return json.dumps(m).encode() if changed else bir_json_bytes


def _patch_nc(nc):
    orig = nc.to_json_bytes
    nc.to_json_bytes = lambda: _split_multiwait(orig())
    return nc


# ---------------------------------------------------------------------------
# Host preprocessing
def _sigmoid(x):
    return 1.0 / (1.0 + np.exp(-x))


def _softplus(x):
    return np.logaddexp(0.0, x)


def _bf(a):
    return np.ascontiguousarray(a).astype(BF16NP)


def _f8(a, scale=QK_SC):
    return np.ascontiguousarray(
        np.clip(np.asarray(a, np.float32) * scale, -240, 240)).astype(E4NP)


def host_prep(inputs):
    f32 = np.float32
    g = {k: np.asarray(v) for k, v in inputs.items()}

    shared = {}
    wqkvT = g["Wqkv"].astype(f32).T                           # (1536, 4608)
    shared["wqkT8"] = _f8(wqkvT[:, :2 * DIM])                 # (1536, 3072) fp8
    shared["wvT"] = _bf(wqkvT[:, 2 * DIM:])                   # (1536, 1536)
    shared["bqkv_pt"] = np.ascontiguousarray(
        g["bqkv"].astype(f32).reshape(36, P).T)               # (128, 36)
    shared["wpj8"] = _f8(g["Wproj"].astype(f32).T)            # (1536, 1536) fp8
    shared["w1T"] = _bf(g["W1"].astype(f32).T)                # (1536, 6144)
    shared["b1_pt"] = np.ascontiguousarray(
        g["b1"].astype(f32).reshape(OT1, P).T)                # (128, 48)
    shared["w2T"] = _bf(g["W2"].astype(f32).T)                # (6144, 1536)
    wT = g["w"].astype(f32).T                                 # (128, 64)
    shared["wT"] = _bf(wT)
    # row-swapped variant for khT's [gate; code] feature order
    shared["wTs"] = _bf(np.concatenate([wT[64:], wT[:64]], axis=0))

    # ---- SPE folds (float64 internally, tiny tensors) ----
    f = _sigmoid(g["spe_freqs"].astype(np.float64)) * 0.5     # (H,D,S)
    off = g["spe_offsets"].astype(np.float64)                 # (H,D,S)
    gains = _softplus(g["spe_gains"].astype(np.float64))      # (H,D,S)
    gate = _sigmoid(g["spe_gate"].astype(np.float64))         # (H,D)
    zn = g["z_noise"].astype(np.float64)[0]                   # (H,D,2S,R)
    gn = g["gating_noise"].astype(np.float64)                 # (H,D,R)

    # zz[h,d,2s+c,r] = z_noise * gains[s]
    zz = zn * np.repeat(gains, 2, axis=-1)[..., None]         # (H,D,2S,R)
    # rotate by offsets so the device only needs cos/sin of the raw phase:
    #   sum_c om_q[2s+c] zz[2s+c] == sum_c om_k[2s+c] zzq[2s+c]
    co = np.cos(off)[..., None]                               # (H,D,S,1)
    so = np.sin(off)[..., None]
    zzq = np.empty_like(zz)
    zzq[:, :, 0::2, :] = co * zz[:, :, 0::2, :] + so * zz[:, :, 1::2, :]
    zzq[:, :, 1::2, :] = -so * zz[:, :, 0::2, :] + co * zz[:, :, 1::2, :]

    scale = (R * D) ** 0.25
    r4 = R ** 0.25
    dn = float(2 * R) ** (-0.25)
    e_fold = (np.sqrt(1.0 - gate) * (dn / (scale * r4)))[:, :, None, None]
    zzq_dev = zzq * e_fold                                    # (H,D,2S,R)
    zzk_dev = zz * e_fold
    gp_dev = np.sqrt(gate)[:, :, None] * gn * (dn / ((D * R) ** 0.25 * r4))

    # device layouts: (H, D, 10*64) with sc-major blocks of 64
    shared["zzq"] = _bf(zzq_dev.reshape(H, D, 2 * S * R))
    shared["zzk"] = _bf(zzk_dev.reshape(H, D, 2 * S * R))
    shared["gp"] = _bf(gp_dev)                                # (H, D, R)

    # trig table om[h, s, d, c*N+n], c=0 cos, c=1 sin
    n_idx = np.arange(N, dtype=np.float64)
    om = np.empty((H, S, D, 2 * N), dtype=BF16NP)
    for s in range(S):
        ph = 2.0 * math.pi * f[:, :, s:s + 1] * n_idx[None, None, :]  # (H,D,N)
        om[:, s, :, :N] = np.cos(ph).astype(BF16NP)
        om[:, s, :, N:] = np.sin(ph).astype(BF16NP)
    shared["om"] = om

    # flags for build specialization (all hold for the spec's fills)
    ones = lambda a: bool(np.all(np.asarray(a) == 1.0))
    zeros = lambda a: bool(np.all(np.asarray(a) == 0.0))
    flags = dict(
        trivial_ln=(ones(g["ln0_g"]) and zeros(g["ln0_b"]) and
                    ones(g["lna_g"]) and zeros(g["lna_b"]) and
                    ones(g["ln2_g"]) and zeros(g["ln2_b"])),
        zero_bv=zeros(g["bqkv"][3072:]),
        zero_bproj=zeros(g["bproj"]),
        zero_b2=zeros(g["b2"]),
    )
    if not flags["trivial_ln"]:
        for nm in ("ln0_g", "ln0_b", "lna_g", "lna_b", "ln2_g", "ln2_b"):
            shared[nm] = np.ascontiguousarray(g[nm].astype(f32)[None, :])
    if not flags["zero_bv"]:
        shared["bv_row"] = np.ascontiguousarray(g["bqkv"].astype(f32)[None, 3072:])
    if not flags["zero_bproj"]:
        shared["bproj_row"] = np.ascontiguousarray(g["bproj"].astype(f32)[None, :])
    if not flags["zero_b2"]:
        shared["b2_row"] = np.ascontiguousarray(g["b2"].astype(f32)[None, :])

    x = np.ascontiguousarray(g["x"].astype(f32))              # (B, N, DIM)
    return shared, x, flags


# ---------------------------------------------------------------------------
_NC_CACHE = {}

_SHAPES = dict(
    x=((N, DIM), F32),
    wqkT8=((DIM, 2 * DIM), FP8),
    wvT=((DIM, DIM), BF16),
    bqkv_pt=((P, 36), F32),
    wpj8=((DIM, DIM), FP8),
    w1T=((DIM, FC1), BF16),
    b1_pt=((P, OT1), F32),
    w2T=((FC1, DIM), BF16),
    wT=((P, M), BF16),
    wTs=((P, M), BF16),
    zzq=((H, D, 2 * S * R), BF16),
    zzk=((H, D, 2 * S * R), BF16),
    gp=((H, D, R), BF16),
    om=((H, S, D, 2 * N), BF16),
    ln0_g=((1, DIM), F32), ln0_b=((1, DIM), F32),
    lna_g=((1, DIM), F32), lna_b=((1, DIM), F32),
    ln2_g=((1, DIM), F32), ln2_b=((1, DIM), F32),
    bv_row=((1, DIM), F32),
    bproj_row=((1, DIM), F32),
    b2_row=((1, DIM), F32),
)


def _input_names(flags):
    names = ["x", "wqkT8", "wvT", "bqkv_pt", "wpj8", "w1T", "b1_pt", "w2T",
             "wT", "wTs", "zzq", "zzk", "gp", "om"]
    if not flags["trivial_ln"]:
        names += ["ln0_g", "ln0_b", "lna_g", "lna_b", "ln2_g", "ln2_b"]
    if not flags["zero_bv"]:
        names.append("bv_row")
    if not flags["zero_bproj"]:
        names.append("bproj_row")
    if not flags["zero_b2"]:
        names.append("b2_row")
    return names


def build_nc(flags, dbg=(), trace_sim=False):
    nc = bass.Bass("TRN2", debug=False)
    ins = {}
    for name in _input_names(flags):
        shp, dt = _SHAPES[name]
        ins[name] = nc.dram_tensor(name, shp, dt, kind="ExternalInput").ap()
    outs = {"out": nc.dram_tensor("out", (N, DIM), F32, kind="ExternalOutput").ap()}
    with tile.TileContext(nc, trace_sim=trace_sim) as tc:
        emit(tc, outs, ins, flags)
    return _patch_nc(nc)


# ---------------------------------------------------------------------------
# Runner
def _run(nc, in_maps):
    from concourse import bass_utils
    return bass_utils.run_bass_kernel_spmd(nc, in_maps, core_ids=list(range(B)))


def get_nc(flags, dbg=()):
    key = (tuple(sorted(flags.items())), tuple(sorted(dbg)))
    if key not in _NC_CACHE:
        _NC_CACHE[key] = build_nc(flags, dbg)
    return _NC_CACHE[key]


def kernel(**inputs):
    shared, x, flags = host_prep(inputs)
    nc = get_nc(flags)
    in_maps = [dict(shared, x=np.ascontiguousarray(x[c])) for c in range(B)]
    res = _run(nc, in_maps)
    out = np.stack([res.results[c]["out"] for c in range(B)], axis=0)
    return out.astype(np.float32)


# ===========================================================================
# The device program
# ===========================================================================
def _ln_pass(nc, sp, in_ap, out_ap, eps_t, gb=None):
    """General single LayerNorm on one (P, DIM) tile via bn_stats + ACT."""
    st6 = sp.tile([P, 3 * 6], F32, tag="ln_st6", name="ln_st6")
    for gi in range(3):
        nc.vector.bn_stats(st6[:, gi * 6:(gi + 1) * 6],
                           in_ap[:, gi * 512:(gi + 1) * 512])
    mv = sp.tile([P, 2], F32, tag="ln_mv", name="ln_mv")
    nc.vector.bn_aggr(mv[:], st6[:].rearrange("p (g s) -> p g s", s=6))
    sd = sp.tile([P, 1], F32, tag="ln_sd", name="ln_sd")
    nc.scalar.activation(sd[:], mv[:, 1:2], AF.Sqrt, bias=eps_t[:], scale=1.0)
    rstd = sp.tile([P, 1], F32, tag="ln_rstd", name="ln_rstd")
    nc.vector.reciprocal(rstd[:], sd[:])
    nbias = sp.tile([P, 1], F32, tag="ln_nb", name="ln_nb")
    nc.vector.scalar_tensor_tensor(nbias[:], in0=mv[:, 0:1], scalar=-1.0,
                                   in1=rstd[:], op0=ALU.mult, op1=ALU.mult)
    if gb is None:
        nc.scalar.activation(out_ap, in_ap, AF.Identity,
                             bias=nbias[:], scale=rstd[:])
    else:
        g_b, b_b = gb
        tmp = sp.tile([P, DIM], F32, tag="ln_tmp", name="ln_tmp")
        nc.scalar.activation(tmp[:], in_ap, AF.Identity,
                             bias=nbias[:], scale=rstd[:])
        nc.vector.tensor_tensor(tmp[:], tmp[:], g_b[:], ALU.mult)
        nc.vector.tensor_tensor(out_ap, tmp[:], b_b[:], ALU.add)


def _ln_double_fused(nc, sp, in_ap, out_ap, eps2_t):
    """LN(LN(x)) with unit gain / zero bias, in one apply pass.

    y = (x-mu)*r1 has exact zero mean; var(y) = v1*r1^2, so the combined
    scale is rsqrt(v1*(1+eps) + eps^2)."""
    st6 = sp.tile([P, 3 * 6], F32, tag="ln_st6", name="ln_st6")
    for gi in range(3):
        nc.vector.bn_stats(st6[:, gi * 6:(gi + 1) * 6],
                           in_ap[:, gi * 512:(gi + 1) * 512])
    mv = sp.tile([P, 2], F32, tag="ln_mv", name="ln_mv")
    nc.vector.bn_aggr(mv[:], st6[:].rearrange("p (g s) -> p g s", s=6))
    sd = sp.tile([P, 1], F32, tag="ln_sd", name="ln_sd")
    nc.scalar.activation(sd[:], mv[:, 1:2], AF.Sqrt, bias=eps2_t[:],
                         scale=1.0 + EPS_LN)
    s_t = sp.tile([P, 1], F32, tag="ln_s", name="ln_s")
    nc.vector.reciprocal(s_t[:], sd[:])
    nbias = sp.tile([P, 1], F32, tag="ln_nb", name="ln_nb")
    nc.vector.scalar_tensor_tensor(nbias[:], in0=mv[:, 0:1], scalar=-1.0,
                                   in1=s_t[:], op0=ALU.mult, op1=ALU.mult)
    nc.scalar.activation(out_ap, in_ap, AF.Identity, bias=nbias[:], scale=s_t[:])


def emit(tc, outs, ins, flags):
    from contextlib import ExitStack
    nc = tc.nc
    trivial_ln = flags["trivial_ln"]

    with ExitStack() as ctx:
        const = ctx.enter_context(tc.tile_pool(name="const", bufs=1))
        sp = ctx.enter_context(tc.tile_pool(name="smalls", bufs=4))
        rp = ctx.enter_context(tc.tile_pool(name="rp", bufs=1))

        eye_bf = const.tile([P, P], BF16, tag="eye_bf", name="eye_bf")
        make_identity(nc, eye_bf[:])
        eye_f = const.tile([P, P], F32, tag="eye_f", name="eye_f")
        make_identity(nc, eye_f[:])
        ones_col = const.tile([P, 1], BF16, tag="ones_col", name="ones_col")
        nc.vector.memset(ones_col[:], 1.0)
        eps_t = const.tile([P, 1], F32, tag="eps_t", name="eps_t")
        nc.vector.memset(eps_t[:], EPS_LN)
        eps2_t = const.tile([P, 1], F32, tag="eps2_t", name="eps2_t")
        nc.vector.memset(eps2_t[:], EPS_LN * EPS_LN)
        ones_row = const.tile([1, P], F32, tag="ones_row", name="ones_row")
        nc.vector.memset(ones_row[:], 1.0)
        bqkv_pt = const.tile([P, 36], F32, tag="bqkv_pt", name="bqkv_pt")
        nc.scalar.dma_start(bqkv_pt[:], ins["bqkv_pt"])
        b1_pt = const.tile([P, OT1], F32, tag="b1_pt", name="b1_pt")
        nc.scalar.dma_start(b1_pt[:], ins["b1_pt"])
        wT_sb = const.tile([P, M], BF16, tag="wT", name="wT")
        nc.scalar.dma_start(wT_sb[:], ins["wT"])
        wTs_sb = const.tile([P, M], BF16, tag="wTs", name="wTs")
        nc.scalar.dma_start(wTs_sb[:], ins["wTs"])

        def bcast_row(name, tag):
            row = const.tile([1, DIM], F32, tag=tag + "_r")
            nc.sync.dma_start(row[:], ins[name])
            t = const.tile([P, DIM], F32, tag=tag)
            nc.gpsimd.partition_broadcast(t[:], row[:])
            return t

        gb0 = gba = gb2 = None
        if not trivial_ln:
            gb0 = (bcast_row("ln0_g", "g0"), bcast_row("ln0_b", "b0"))
            gba = (bcast_row("lna_g", "ga"), bcast_row("lna_b", "ba"))
            gb2 = (bcast_row("ln2_g", "g2"), bcast_row("ln2_b", "b2"))
        bv_b = None if flags["zero_bv"] else bcast_row("bv_row", "bv")
        bproj_b = None if flags["zero_bproj"] else bcast_row("bproj_row", "bpj")
        b2_b = None if flags["zero_b2"] else bcast_row("b2_row", "b2v")

        # residual stream as one tile (128, NT, DIM); x tiles stream in
        # interleaved with the stage-A transposes on the sync queue (below)
        r1 = rp.tile([P, NT * DIM], F32, tag="r1", name="r1")
        r1_3 = r1[:].rearrange("p (t f) -> p t f", f=DIM)

        def load_x(t):
            nc.sync.dma_start(r1_3[:, t, :], ins["x"][t * P:(t + 1) * P, :])

        def rsl(t):
            return r1[:, t * DIM:(t + 1) * DIM]

        # pools alive through stage A + heads
        with tc.tile_pool(name="htp", bufs=1) as htp, \
             tc.tile_pool(name="vtp", bufs=1) as vtp, \
             tc.tile_pool(name="ytp", bufs=1) as ytp:
            hT8 = htp.tile([P, DC * N], FP8, tag="hT8", name="hT8")
            hT8_3 = hT8[:].rearrange("p (d n) -> p d n", n=N)
            v_tok = [vtp.tile([P, DIM], BF16, tag=f"v{t}", name=f"v{t}")
                     for t in range(NT)]
            yT = ytp.tile([P, H * N], FP8, tag="yT", name="yT")
            yT3 = yT[:].rearrange("p (d n) -> p d n", n=N)

            # ---------------- Stage A: LN + transpose + v --------------------
            with tc.tile_pool(name="lnp", bufs=3) as lnp, \
                 tc.tile_pool(name="wvp", bufs=1) as wvp, \
                 tc.tile_pool(name="psA", bufs=3, space="PSUM") as psA:
                hT = wvp.tile([P, DC * N], BF16, tag="hT", name="hT")
                hT3 = hT[:].rearrange("p (d n) -> p d n", n=N)
                # All stage-A DMAs are input-ready at t=0, so the sync queue
                # pops them in program order: interleave x tiles with wv
                # chunks so LN(0) and v(0) start earliest.
                wv = wvp.tile([P, DC * DIM], BF16, tag="wv", name="wv")
                wv3 = wv[:].rearrange("p (d f) -> p d f", f=DIM)

                def load_wv(oc):
                    nc.sync.dma_start(
                        wv3[:, :, oc * 512:(oc + 1) * 512],
                        ins["wvT"][:, oc * 512:(oc + 1) * 512]
                        .rearrange("(d p) f -> p d f", p=P))

                load_x(0)
                load_wv(0)
                load_x(1)
                load_wv(1)
                load_x(2)
                load_wv(2)
                for t in range(3, NT):
                    load_x(t)
                for t in range(NT):
                    htok = lnp.tile([P, DIM], BF16, tag="ln_h", name="ln_h")
                    if trivial_ln:
                        _ln_double_fused(nc, lnp, rsl(t), htok[:], eps2_t)
                    else:
                        ytmp = lnp.tile([P, DIM], BF16, tag="ln_y",
                                        name="ln_y")
                        _ln_pass(nc, lnp, rsl(t), ytmp[:], eps_t, gb=gb0)
                        _ln_pass(nc, lnp, ytmp[:], htok[:], eps_t, gb=gba)
                    # transposes on the scalar hwdge queue: they become ready
                    # right after apply(t) and interleave with the LN ops
                    nc.scalar.dma_start_transpose(
                        hT3[:, :, t * P:(t + 1) * P], htok[:])
                    # fp8 copy of h^T feeds the DoubleRow qk matmuls
                    nc.vector.tensor_copy(hT8_3[:, :, t * P:(t + 1) * P],
                                          hT3[:, :, t * P:(t + 1) * P])
                    for oc in range(3):
                        psv = psA.tile([P, 512], F32, tag="psA", name="psA")
                        for d in range(DC):
                            nc.tensor.matmul(
                                psv[:],
                                hT[:, d * N + t * P:d * N + (t + 1) * P],
                                wv[:, d * DIM + oc * 512:
                                   d * DIM + (oc + 1) * 512],
                                start=(d == 0), stop=(d == DC - 1))
                        nc.vector.tensor_copy(
                            v_tok[t][:, oc * 512:(oc + 1) * 512], psv[:])
                    if bv_b is not None:
                        nc.vector.tensor_tensor(v_tok[t][:], v_tok[t][:],
                                                bv_b[:], ALU.add)
                    # residual init r1 += vf
                    nc.vector.tensor_tensor(rsl(t), rsl(t), v_tok[t][:],
                                            ALU.add)

            # ---------------- Heads: software-pipelined ----------------------
            with tc.tile_pool(name="hqk", bufs=2) as hqk, \
                 tc.tile_pool(name="spep", bufs=2) as spep, \
                 tc.tile_pool(name="omp", bufs=3) as omp, \
                 tc.tile_pool(name="gqk", bufs=3) as gqk, \
                 tc.tile_pool(name="wqp", bufs=3) as wqp, \
                 tc.tile_pool(name="fav", bufs=2) as fav, \
                 tc.tile_pool(name="psQK", bufs=2, space="PSUM") as psQK, \
                 tc.tile_pool(name="psSPE", bufs=2, space="PSUM") as psSPE, \
                 tc.tile_pool(name="psDG", bufs=2, space="PSUM") as psDG, \
                 tc.tile_pool(name="psm", bufs=2, space="PSUM") as psm:

                def bc(dst, src_row, w):
                    bps = psm.tile([P, w], F32, tag="sps", name="bcps")
                    nc.tensor.matmul(bps[:], ones_row[:], src_row, start=True,
                                     stop=True)
                    nc.scalar.activation(dst, bps[:], AF.Copy, scale=1.0)

                st = {}

                def block_A(hh):
                    """qT/kT for head hh (feature-major, (128, N) each).

                    fp8 DoubleRow: 6 matmuls of K=256 instead of 12 of K=128;
                    the 1/QK_SC weight descale folds into the psum copy."""
                    s = st.setdefault(hh, {})
                    qT = hqk.tile([P, N], BF16, tag="qT", name="qT")
                    kT = hqk.tile([P, N], BF16, tag="kT", name="kT")
                    s["qT"], s["kT"] = qT, kT
                    for oi, (o, dst) in enumerate(((hh, qT), (H + hh, kT))):
                        wt = wqp.tile([P, DC * P], FP8, tag="wt", name="wt")
                        nc.sync.dma_start(
                            wt[:].rearrange("p (dc c) -> p dc c", c=P),
                            ins["wqkT8"][:, o * P:(o + 1) * P]
                            .rearrange("(dc p) c -> p dc c", p=P))
                        wt3 = wt[:].rearrange("p (dc c) -> p dc c", c=P)
                        for hf in range(2):
                            pst = psQK.tile([P, 512], F32, tag="psQK",
                                            name="psQK")
                            for dp in range(DC // 2):
                                nc.tensor.matmul(
                                    pst[:],
                                    wt3[:, 2 * dp:2 * dp + 2, :],
                                    hT8_3[:, 2 * dp:2 * dp + 2,
                                          hf * 512:hf * 512 + 512],
                                    start=(dp == 0), stop=(dp == DC // 2 - 1),
                                    perf_mode=mybir.MatmulPerfMode.DoubleRow)
                            nc.scalar.activation(
                                dst[:, hf * 512:(hf + 1) * 512], pst[:],
                                AF.Identity, bias=bqkv_pt[:, o:o + 1],
                                scale=1.0 / QK_SC)

                def block_S(hh):
                    """SPE filter -> qhT, khT (feature-major (128, N)).

                    The q-code matmul (psum rows 0:64, array col groups 0-1)
                    and k-code matmul (rows 64:128, col groups 2-3) are issued
                    adjacently with different moving operands, so the PE runs
                    them concurrently on HW (col-tiled). 2 PSUM banks live."""
                    s = st[hh]
                    qT, kT = s["qT"], s["kT"]
                    zzq_t = spep.tile([P, 2 * S * R], BF16, tag="zzq", name="zzq")
                    nc.sync.dma_start(zzq_t[:], ins["zzq"][hh])
                    zzk_t = spep.tile([P, 2 * S * R], BF16, tag="zzk", name="zzk")
                    nc.sync.dma_start(zzk_t[:], ins["zzk"][hh])
                    gp_t = spep.tile([P, R], BF16, tag="gp", name="gp")
                    nc.sync.dma_start(gp_t[:], ins["gp"][hh])
                    qhT = spep.tile([P, N], BF16, tag="qhT", name="qhT")
                    khT = spep.tile([P, N], BF16, tag="khT", name="khT")
                    s["qhT"], s["khT"] = qhT, khT
                    # psum A: [q-code 0:64 ; k-code 64:128]  (col grps 0-1 / 2-3)
                    # psum B: [k-gate 0:64 ; q-gate 64:128]
                    # => khT feature order is [gate; code] (dd_k uses wTs_sb,
                    # the row-swapped wT); all psum->SBUF copies stay
                    # partition-aligned.
                    ps_code = [psSPE.tile([P, 512], F32, tag="psSPE",
                                          name="psSPE") for _ in range(2)]
                    for s5 in range(S):
                        om_t = omp.tile([P, 2 * N], BF16, tag="om", name="om")
                        nc.sync.dma_start(om_t[:], ins["om"][hh, s5])
                        # one wide product per side covers both c (cos/sin):
                        # G[p, c*N+n] = om[p, c*N+n] * qT[p, n]
                        om3 = om_t[:].rearrange("p (c n) -> p c n", n=N)
                        Gq = gqk.tile([P, 2 * N], BF16, tag="Gq", name="Gq")
                        nc.vector.tensor_tensor(
                            Gq[:].rearrange("p (c n) -> p c n", n=N), om3,
                            qT[:].rearrange("p (c n) -> p c n", c=1)
                            .broadcast_to([P, 2, N]), ALU.mult)
                        Gk = gqk.tile([P, 2 * N], BF16, tag="Gk", name="Gk")
                        nc.vector.tensor_tensor(
                            Gk[:].rearrange("p (c n) -> p c n", n=N), om3,
                            kT[:].rearrange("p (c n) -> p c n", c=1)
                            .broadcast_to([P, 2, N]), ALU.mult)
                        for c in range(2):
                            sc = 2 * s5 + c
                            for hf in range(2):
                                nc.tensor.matmul(
                                    ps_code[hf][0:64, :],
                                    zzq_t[:, sc * R:(sc + 1) * R],
                                    Gq[:, c * N + hf * 512:c * N + hf * 512 + 512],
                                    start=(sc == 0), stop=(sc == 2 * S - 1),
                                    skip_group_check=True)
                                nc.tensor.matmul(
                                    ps_code[hf][64:128, :],
                                    zzk_t[:, sc * R:(sc + 1) * R],
                                    Gk[:, c * N + hf * 512:c * N + hf * 512 + 512],
                                    start=(sc == 0), stop=(sc == 2 * S - 1),
                                    skip_group_check=True)
                    ps_gate = [psSPE.tile([P, 512], F32, tag="psSPE",
                                          name="psSPE") for _ in range(2)]
                    for hf in range(2):
                        nc.tensor.matmul(ps_gate[hf][0:64, :], gp_t[:],
                                         kT[:, hf * 512:(hf + 1) * 512],
                                         start=True, stop=True,
                                         skip_group_check=True)
                        nc.tensor.matmul(ps_gate[hf][64:128, :], gp_t[:],
                                         qT[:, hf * 512:(hf + 1) * 512],
                                         start=True, stop=True,
                                         skip_group_check=True)
                    for hf in range(2):
                        nc.scalar.activation(qhT[0:64, hf * 512:(hf + 1) * 512],
                                             ps_code[hf][0:64, :], AF.Copy,
                                             scale=1.0)
                        nc.scalar.activation(khT[64:128, hf * 512:(hf + 1) * 512],
                                             ps_code[hf][64:128, :], AF.Copy,
                                             scale=1.0)
                        nc.scalar.activation(qhT[64:128, hf * 512:(hf + 1) * 512],
                                             ps_gate[hf][64:128, :], AF.Copy,
                                             scale=1.0)
                        nc.scalar.activation(khT[0:64, hf * 512:(hf + 1) * 512],
                                             ps_gate[hf][0:64, :], AF.Copy,
                                             scale=1.0)

                def block_K1(hh):
                    """k-side dd + diag + running max (PE small + chains)."""
                    s = st[hh]
                    khT = s["khT"]
                    dd_k = fav.tile([P, NT * M], F32, tag="ddk", name="ddk")
                    diag_k = fav.tile([P, NT], F32, tag="dgk", name="dgk")
                    s["dd_k"], s["diag_k"] = dd_k, diag_k
                    for t in range(NT):
                        # dd (cols 0:64) and gram (cols 64:192) share one
                        # psum tile so each token tile costs one rotation slot
                        dg = psDG.tile([P, M + P], F32, tag="ddgr", name="ddgrk")
                        nc.tensor.matmul(dg[:, 0:M], khT[:, t * P:(t + 1) * P],
                                         wTs_sb[:], start=True, stop=True,
                                         skip_group_check=True)
                        nc.tensor.matmul(dg[:, M:M + P],
                                         khT[:, t * P:(t + 1) * P],
                                         khT[:, t * P:(t + 1) * P],
                                         start=True, stop=True,
                                         skip_group_check=True)
                        nc.scalar.activation(dd_k[:, t * M:(t + 1) * M],
                                             dg[:, 0:M], AF.Copy, scale=1.0)
                        scr = sp.tile([P, P], BF16, tag="scr128", name="scr128")
                        nc.vector.scalar_tensor_tensor(
                            scr[:], in0=dg[:, M:M + P], scalar=0.5, in1=eye_f[:],
                            op0=ALU.mult, op1=ALU.mult,
                            accum_out=diag_k[:, t:t + 1])
                    # one reduce over all 8 tiles' dd, then partition max
                    mxk = fav.tile([P, 1], F32, tag="mxk", name="mxk")
                    nc.vector.tensor_reduce(mxk[:], dd_k[:], AX.X, ALU.max)
                    mxk1 = sp.tile([1, 1], F32, tag="mxk1", name="mxk1")
                    nc.gpsimd.tensor_reduce(mxk1[:], mxk[:], AX.C, ALU.max)
                    s["mxk1"] = mxk1

                def block_Q1(hh):
                    """q-side dd/diag matmuls + bias/exp chains."""
                    s = st[hh]
                    qhT = s["qhT"]
                    qp_raw = [fav.tile([P, M], F32, tag=f"qpr{t}", name=f"qpr{t}")
                              for t in range(NT)]
                    s["qp_raw"] = qp_raw
                    for t in range(NT):
                        dg = psDG.tile([P, M + P], F32, tag="ddgr", name="ddgrq")
                        nc.tensor.matmul(dg[:, 0:M], qhT[:, t * P:(t + 1) * P],
                                         wT_sb[:], start=True, stop=True,
                                         skip_group_check=True)
                        nc.tensor.matmul(dg[:, M:M + P],
                                         qhT[:, t * P:(t + 1) * P],
                                         qhT[:, t * P:(t + 1) * P],
                                         start=True, stop=True,
                                         skip_group_check=True)
                        negmx = sp.tile([P, 1], F32, tag="negmx", name="negmx")
                        nc.vector.tensor_reduce(negmx[:], dg[:, 0:M], AX.X,
                                                ALU.max, negate=True)
                        biasq = sp.tile([P, 1], F32, tag="biasq", name="biasq")
                        scr = sp.tile([P, P], BF16, tag="scr128", name="scr128")
                        nc.vector.scalar_tensor_tensor(
                            scr[:], in0=dg[:, M:M + P], scalar=-0.5, in1=eye_f[:],
                            op0=ALU.mult, op1=ALU.mult, accum_out=biasq[:])
                        nc.gpsimd.tensor_tensor(biasq[:], biasq[:], negmx[:],
                                                ALU.add)
                        nc.scalar.activation(qp_raw[t][:], dg[:, 0:M], AF.Exp,
                                             bias=biasq[:], scale=1.0)

                def block_K2a(hh):
                    """broadcast global k max; kp chains."""
                    s = st[hh]
                    mxkb = fav.tile([P, 1], F32, tag="mxkb", name="mxkb")
                    bc(mxkb[:], s["mxk1"][:], 1)
                    kp = [fav.tile([P, M], BF16, tag=f"kp{t}", name=f"kp{t}")
                          for t in range(NT)]
                    s["kp"] = kp
                    for t in range(NT):
                        biask = sp.tile([P, 1], F32, tag="biask", name="biask")
                        nc.vector.scalar_tensor_tensor(
                            biask[:], in0=s["diag_k"][:, t:t + 1], scalar=-1.0,
                            in1=mxkb[:], op0=ALU.mult, op1=ALU.subtract)
                        kpf = sp.tile([P, M], F32, tag="kpf", name="kpf")
                        nc.scalar.activation(kpf[:],
                                             s["dd_k"][:, t * M:(t + 1) * M],
                                             AF.Exp, bias=biask[:], scale=1.0)
                        nc.gpsimd.tensor_scalar(out=kp[t][:], in0=kpf[:],
                                                scalar1=EPS_K, scalar2=None,
                                                op0=ALU.add)

                def block_K2b(hh):
                    """ksum, seps, ctx (PE + small chains)."""
                    s = st[hh]
                    kp = s["kp"]
                    ksps = psm.tile([1, M], F32, tag="sps", name="ksps")
                    for t in range(NT):
                        nc.tensor.matmul(ksps[:], ones_col[:], kp[t][:],
                                         start=(t == 0), stop=(t == NT - 1))
                    ksrow = sp.tile([1, M], F32, tag="ksrow", name="ksrow")
                    nc.scalar.activation(ksrow[:], ksps[:], AF.Copy, scale=1.0)
                    ksum_b = fav.tile([P, M], F32, tag="ksum_b", name="ksum_b")
                    bc(ksum_b[:], ksrow[:], M)
                    kss = sp.tile([1, 1], F32, tag="kss", name="kss")
                    nc.vector.tensor_reduce(kss[:], ksrow[:], AX.X, ALU.add)
                    seps_r = sp.tile([1, 1], F32, tag="seps_r", name="seps_r")
                    nc.vector.tensor_scalar(out=seps_r[:], in0=kss[:],
                                            scalar1=8.0 * EPS_K, scalar2=None,
                                            op0=ALU.mult)
                    seps8 = fav.tile([P, 1], F32, tag="seps8", name="seps8")
                    bc(seps8[:], seps_r[:], 1)
                    s["ksum_b"], s["seps8"] = ksum_b, seps8
                    cps = psm.tile([M, P], F32, tag="sps", name="ctxp")
                    for t in range(NT):
                        nc.tensor.matmul(cps[:], kp[t][:],
                                         v_tok[t][:, hh * P:(hh + 1) * P],
                                         start=(t == 0), stop=(t == NT - 1))
                    ctx_sb = fav.tile([M, P], BF16, tag="ctx_sb", name="ctx_sb")
                    nc.scalar.activation(ctx_sb[:], cps[:], AF.Copy, scale=1.0)
                    s["ctx_sb"] = ctx_sb

                def block_Q2(hh):
                    """den/dinv chains, batched qp transpose, y accumulation."""
                    s = st[hh]
                    qp_raw, ksum_b, seps8 = s["qp_raw"], s["ksum_b"], s["seps8"]
                    ctx_sb = s["ctx_sb"]
                    dinv8s = []
                    for t in range(NT):
                        den8 = sp.tile([P, 1], F32, tag="den8", name="den8")
                        scr64 = sp.tile([P, M], BF16, tag="scr64", name="scr64")
                        nc.vector.scalar_tensor_tensor(
                            scr64[:], in0=qp_raw[t][:], scalar=8.0, in1=ksum_b[:],
                            op0=ALU.mult, op1=ALU.mult, accum_out=den8[:])
                        nc.gpsimd.tensor_tensor(den8[:], den8[:], seps8[:],
                                                ALU.add)
                        dinv8 = sp.tile([P, 1], F32, tag="dinv8", name="dinv8")
                        nc.vector.reciprocal(dinv8[:], den8[:])
                        dinv8s.append(dinv8)
                    for t in range(NT):
                        qp_hat = sp.tile([P, M], BF16, tag="qp_hat", name="qp_hat")
                        nc.vector.tensor_scalar(out=qp_hat[:], in0=qp_raw[t][:],
                                                scalar1=EPS_K,
                                                scalar2=dinv8s[t][:],
                                                op0=ALU.add, op1=ALU.mult)
                        tps = psm.tile([M, P], BF16, tag="sps", name="tpsp")
                        nc.tensor.transpose(tps[:], qp_hat[:], eye_bf[:])
                        qpT_sb = sp.tile([M, P], BF16, tag="qpT_sb", name="qpT_sb")
                        nc.scalar.activation(qpT_sb[:], tps[:], AF.Copy,
                                             scale=1.0)
                        yps = psm.tile([P, P], F32, tag="sps", name="ypsp")
                        nc.tensor.matmul(yps[:], ctx_sb[:], qpT_sb[:],
                                         start=True, stop=True)
                        nc.scalar.activation(
                            yT[:, hh * N + t * P:hh * N + (t + 1) * P], yps[:],
                            AF.Copy, scale=1.0)

                # software pipeline over heads
                block_A(0)
                block_S(0)
                for hh in range(H):
                    block_K1(hh)
                    block_Q1(hh)
                    if hh + 1 < H:
                        block_A(hh + 1)
                    block_K2a(hh)
                    block_K2b(hh)
                    if hh + 1 < H:
                        block_S(hh + 1)
                    block_Q2(hh)
                    st.pop(hh)

            # ---------------- proj + residual --------------------------------
            # t-outer so r1[t] completes early and LN2 can chase proj;
            # fp8 DoubleRow over 6 head-pairs, descale in the residual add
            with tc.tile_pool(name="wpp", bufs=1) as wpp, \
                 tc.tile_pool(name="psE", bufs=3, space="PSUM") as psE:
                wp = wpp.tile([P, DC * DIM], FP8, tag="wp", name="wp")
                wp3 = wp[:].rearrange("p (d c) -> p d c", c=DIM)
                for oc in range(3):
                    nc.sync.dma_start(
                        wp3[:, :, oc * 512:(oc + 1) * 512],
                        ins["wpj8"][:, oc * 512:(oc + 1) * 512]
                        .rearrange("(d p) c -> p d c", p=P))
                for t in range(NT):
                    for oc in range(3):
                        pst = psE.tile([P, 512], F32, tag="psE", name="psE")
                        for dp in range(DC // 2):
                            nc.tensor.matmul(
                                pst[:],
                                yT3[:, 2 * dp:2 * dp + 2, t * P:(t + 1) * P],
                                wp3[:, 2 * dp:2 * dp + 2,
                                    oc * 512:oc * 512 + 512],
                                start=(dp == 0), stop=(dp == DC // 2 - 1),
                                perf_mode=mybir.MatmulPerfMode.DoubleRow)
                        sl = r1[:, t * DIM + oc * 512:t * DIM + (oc + 1) * 512]
                        nc.vector.scalar_tensor_tensor(
                            sl, in0=pst[:], scalar=1.0 / QK_SC, in1=sl,
                            op0=ALU.mult, op1=ALU.add)
                        if bproj_b is not None:
                            nc.vector.tensor_tensor(
                                sl, sl, bproj_b[:, oc * 512:(oc + 1) * 512],
                                ALU.add)

        # ---------------- LN2 + MLP + residual ----------------------------
        with tc.tile_pool(name="a1p", bufs=1) as a1p:
            a1T = a1p.tile([P, OT1 * N], BF16, tag="a1T", name="a1T")
            w2p_cm = tc.tile_pool(name="w2p", bufs=5)
            w2p = w2p_cm.__enter__()
            with tc.tile_pool(name="h2tp", bufs=1) as h2tp, \
                 tc.tile_pool(name="ln2p", bufs=3) as ln2p:
                h2T = h2tp.tile([P, DC * N], BF16, tag="h2T", name="h2T")
                h2T3 = h2T[:].rearrange("p (d n) -> p d n", n=N)
                for t in range(NT):
                    h2 = ln2p.tile([P, DIM], BF16, tag="h2", name="h2")
                    _ln_pass(nc, ln2p, rsl(t), h2[:], eps_t, gb=gb2)
                    nc.scalar.dma_start_transpose(
                        h2T3[:, :, t * P:(t + 1) * P], h2[:])

                HO = OT1 // 4     # 12 oc per quarter-chunk

                def load_w2(w2d, dt, half):
                    nc.scalar.dma_start(
                        w2d[:].rearrange("p (oc c) -> p oc c", c=P),
                        ins["w2T"][half * HO * P:(half + 1) * HO * P,
                                   dt * P:(dt + 1) * P]
                        .rearrange("(oc p) c -> p oc c", p=P))

                # fc1 + gelu -> a1T feature-major; dt=0's fc2 weights prefetch
                # during fc1 so fc2 starts without a DMA wait
                w2h0 = []
                with tc.tile_pool(name="w1p", bufs=2) as w1p, \
                     tc.tile_pool(name="psF1", bufs=3, space="PSUM") as psF1:
                    for half in range(4):
                        w2d = w2p.tile([P, HO * P], BF16, tag="w2d", name="w2d")
                        load_w2(w2d, 0, half)
                        w2h0.append(w2d)
                    for o in range(OT1):
                        wt = w1p.tile([P, DC * P], BF16, tag="w1t", name="w1t")
                        nc.sync.dma_start(
                            wt[:].rearrange("p (dc c) -> p dc c", c=P),
                            ins["w1T"][:, o * P:(o + 1) * P]
                            .rearrange("(dc p) c -> p dc c", p=P))
                        for hf in range(2):
                            pst = psF1.tile([P, 512], F32, tag="psF1",
                                            name="psF1")
                            for d in range(DC):
                                nc.tensor.matmul(
                                    pst[:], wt[:, d * P:(d + 1) * P],
                                    h2T[:, d * N + hf * 512:d * N + hf * 512 + 512],
                                    start=(d == 0), stop=(d == DC - 1))
                            nc.scalar.activation(
                                a1T[:, o * N + hf * 512:o * N + hf * 512 + 512],
                                pst[:], AF.Gelu, bias=b1_pt[:, o:o + 1],
                                scale=1.0)

            # fc2: feature-major out, batched transpose, strided add into r1
            with tc.tile_pool(name="f2p", bufs=3) as f2p, \
                 tc.tile_pool(name="psF2", bufs=3, space="PSUM") as psF2:
                for dt in range(DC):
                    if dt == 0:
                        w2h = w2h0
                    else:
                        w2h = []
                        for half in range(4):
                            w2d = w2p.tile([P, HO * P], BF16, tag="w2d",
                                           name="w2d")
                            load_w2(w2d, dt, half)
                            w2h.append(w2d)
                    fT = f2p.tile([P, N], BF16, tag="fT", name="fT")
                    for hf in range(2):
                        pst = psF2.tile([P, 512], F32, tag="psF2", name="psF2")
                        for oc in range(OT1):
                            w2d = w2h[oc // HO]
                            ol = oc % HO
                            nc.tensor.matmul(
                                pst[:], w2d[:, ol * P:(ol + 1) * P],
                                a1T[:, oc * N + hf * 512:oc * N + hf * 512 + 512],
                                start=(oc == 0), stop=(oc == OT1 - 1))
                        nc.scalar.activation(fT[:, hf * 512:(hf + 1) * 512],
                                             pst[:], AF.Copy, scale=1.0)
                    ftok = f2p.tile([P, NT * P], BF16, tag="ftok", name="ftok")
                    nc.sync.dma_start_transpose(
                        ftok[:].rearrange("p (t c) -> p t c", c=P), fT[:])
                    # strided add into r1: r1[:, t, dt*128:+128] += ftok[:, t, :]
                    nc.vector.tensor_tensor(
                        r1_3[:, :, dt * P:(dt + 1) * P],
                        r1_3[:, :, dt * P:(dt + 1) * P],
                        ftok[:].rearrange("p (t c) -> p t c", c=P), ALU.add)
                    if b2_b is None:
                        nc.sync.dma_start(
                            outs["out"].rearrange("(t p) f -> p t f", p=P)
                            [:, :, dt * P:(dt + 1) * P],
                            r1_3[:, :, dt * P:(dt + 1) * P])
                if b2_b is not None:
                    for t in range(NT):
                        nc.vector.tensor_tensor(rsl(t), rsl(t), b2_b[:],
                                                ALU.add)
                        nc.sync.dma_start(
                            outs["out"][t * P:(t + 1) * P, :], rsl(t))
            w2p_cm.__exit__(None, None, None)
